# revision 9
# baseline (speedup 1.0000x reference)
"""Trainium2 Bass kernel for per-sample multi-head Linear (MoE-style routing).

Computes logits[i] = x[i] @ W[system_id[i]].T + b[system_id[i]] for
x:[B,D]=[262144,256], W:[S,C,D]=[16,10,256], b:[S,C], int system ids.

Strategy: data-parallel over 8 NeuronCores (32768 rows each). Each core:
  - streams x^T tiles (host pre-transposed so D sits on SBUF partitions),
  - computes the dense all-system logits [128b, 160sc] with two fp32r
    matmuls (k=128 each) + a rank-1 bias matmul into PSUM,
  - selects each row's own head with a fused (iota==sid)*logits multiply
    on DVE followed by a segmented reduce over the 16 systems,
  - writes [128,10] results back, batched per 1024-row chunk.
"""

import sys
import numpy as np

if "/opt/trn_rl_repo" not in sys.path:
    sys.path.insert(0, "/opt/trn_rl_repo")

import concourse.bacc as bacc
import concourse.bass as bass
import concourse.mybir as mybir
import concourse.tile as tile
from concourse.bass_utils import run_bass_kernel_spmd

B = 262144
D = 256
S = 16
C = 10
N_CORES = 8
B_CORE = B // N_CORES  # 32768

SC = S * C           # 160
SC_PAD = 256         # pad matmul free dim to 256 -> fp32r runs 1 cyc/row
CHUNK_B = 1024       # rows per DMA chunk
SUB_B = 128          # rows per matmul subtile
SUBS = CHUNK_B // SUB_B

F32 = mybir.dt.float32
F32R = mybir.dt.float32r


def build_nc(n_rows: int = B_CORE, chunk_b: int = CHUNK_B):
    """Build the per-core Bass program. Same program runs SPMD on all cores."""
    assert n_rows % chunk_b == 0 and chunk_b % SUB_B == 0
    n_chunks = n_rows // chunk_b
    subs = chunk_b // SUB_B
    n_tiles = n_rows // SUB_B

    nc = bacc.Bacc(
        "TRN2",
        target_bir_lowering=False,
        debug=False,
        num_devices=N_CORES,
    )

    xT = nc.dram_tensor("xT", [D, n_rows], F32R, kind="ExternalInput")
    sid = nc.dram_tensor("sid", [SUB_B, n_tiles], F32, kind="ExternalInput")
    wt = nc.dram_tensor("wt", [D, SC_PAD], F32R, kind="ExternalInput")
    biasrow = nc.dram_tensor("biasrow", [1, SC_PAD], F32R, kind="ExternalInput")
    ones = nc.dram_tensor("ones", [1, SUB_B], F32R, kind="ExternalInput")
    # sfull[p, c*S + s] = s  (iota over systems, repeated per class)
    sfull = nc.dram_tensor("sfull", [SUB_B, C * S], F32, kind="ExternalInput")
    out = nc.dram_tensor("out", [n_rows, C], F32, kind="ExternalOutput")

    with tile.TileContext(nc) as tc:
        with (
            tc.tile_pool(name="consts", bufs=1) as consts,
            tc.tile_pool(name="xtp0", bufs=3) as xtp0,
            tc.tile_pool(name="xtp1", bufs=3) as xtp1,
            tc.tile_pool(name="alp", bufs=4) as alp,
            tc.tile_pool(name="prodp", bufs=4) as prodp,
            tc.tile_pool(name="outp", bufs=3) as outp,
            tc.tile_pool(name="psum", bufs=4, space=bass.MemorySpace.PSUM) as psump,
        ):
            wt0 = consts.tile([SUB_B, SC_PAD], F32R, tag="wt0")
            wt1 = consts.tile([SUB_B, SC_PAD], F32R, tag="wt1")
            bias_t = consts.tile([1, SC_PAD], F32R, tag="bias")
            ones_t = consts.tile([1, SUB_B], F32R, tag="ones")
            sfull_t = consts.tile([SUB_B, C * S], F32, tag="sfull")
            sid_t = consts.tile([SUB_B, n_tiles], F32, tag="sid")

            nc.sync.dma_start(wt0[:], wt[0:SUB_B, :])
            nc.sync.dma_start(wt1[:], wt[SUB_B : 2 * SUB_B, :])
            nc.sync.dma_start(bias_t[:], biasrow[:])
            nc.sync.dma_start(ones_t[:], ones[:])
            nc.sync.dma_start(sfull_t[:], sfull[:])
            nc.sync.dma_start(sid_t[:], sid[:])

            out_r = out.rearrange("(n j p) c -> n p j c", p=SUB_B, j=subs)

            for ci in range(n_chunks):
                xt0 = xtp0.tile([SUB_B, chunk_b], F32R, tag="xt0")
                xt1 = xtp1.tile([SUB_B, chunk_b], F32R, tag="xt1")
                c0 = ci * chunk_b
                nc.sync.dma_start(xt0[:], xT[0:SUB_B, c0 : c0 + chunk_b])
                nc.sync.dma_start(xt1[:], xT[SUB_B : 2 * SUB_B, c0 : c0 + chunk_b])

                outb = outp.tile([SUB_B, subs, C], F32, tag="outb")

                for j in range(subs):
                    t = ci * subs + j
                    ps = psump.tile([SUB_B, SC_PAD], F32, tag="ps")
                    js = j * SUB_B
                    nc.tensor.matmul(
                        ps[:],
                        xt0[:, js : js + SUB_B],
                        wt0[:],
                        start=True,
                        stop=False,
                    )
                    nc.tensor.matmul(
                        ps[:],
                        xt1[:, js : js + SUB_B],
                        wt1[:],
                        start=False,
                        stop=False,
                    )
                    nc.tensor.matmul(
                        ps[:],
                        ones_t[:],
                        bias_t[:],
                        start=False,
                        stop=True,
                    )

                    # ACT copies the 160 real logits out of PSUM.
                    al = alp.tile([SUB_B, SC], F32, tag="al")
                    nc.scalar.copy(al[:], ps[:, 0:SC])

                    # prod[p, c, s] = (sfull[p,c,s] == sid[p,t]) * al[p, s*C + c]
                    prod = prodp.tile([SUB_B, C, S], F32, tag="prod")
                    al_cs = al[:].rearrange("p (s c) -> p c s", s=S, c=C)
                    nc.vector.scalar_tensor_tensor(
                        out=prod[:],
                        in0=sfull_t[:].rearrange("p (c s) -> p c s", c=C, s=S),
                        scalar=sid_t[:, t : t + 1],
                        in1=al_cs,
                        op0=mybir.AluOpType.is_equal,
                        op1=mybir.AluOpType.mult,
                    )
                    # sel[p, c] = sum_s prod[p, c, s]
                    nc.vector.tensor_reduce(
                        out=outb[:, j, :],
                        in_=prod[:],
                        axis=mybir.AxisListType.X,
                        op=mybir.AluOpType.add,
                    )

                nc.sync.dma_start(out_r[ci], outb[:])

    nc.compile()
    return nc


def _round_fp32r(a):
    """Round fp32 -> fp32r (round-to-nearest-even at 13 dropped mantissa bits),
    matching walrus's fp32_to_fp32r. Matmul operands are consumed at this
    precision by the PE, so pre-rounding keeps host/sim/HW consistent."""
    bits = a.astype(np.float32).view(np.uint32)
    lsb = (bits >> np.uint32(13)) & np.uint32(1)
    rounded = (bits + np.uint32(0x0FFF) + lsb) & np.uint32(0xFFFFE000)
    return rounded.view(np.float32)


def _host_prep(x, system_id, W, b):
    """Host-side layout prep shared by all cores (weights) and per-core (x/sid)."""
    wt = np.zeros((D, SC_PAD), dtype=np.float32)
    wt[:, :SC] = _round_fp32r(W.reshape(SC, D).T)
    biasrow = np.zeros((1, SC_PAD), dtype=np.float32)
    biasrow[0, :SC] = _round_fp32r(b.reshape(SC))
    ones = np.ones((1, SUB_B), dtype=np.float32)
    sfull = np.tile(
        np.repeat(np.arange(S, dtype=np.float32)[None, :], C, axis=0).reshape(1, C * S),
        (SUB_B, 1),
    )
    return wt, biasrow, ones, sfull


_NC_CACHE = {}


def kernel(x, system_id, W, b):
    x = np.asarray(x, dtype=np.float32)
    system_id = np.asarray(system_id)
    W = np.asarray(W, dtype=np.float32)
    b = np.asarray(b, dtype=np.float32)

    key = (x.shape[0],)
    if key not in _NC_CACHE:
        _NC_CACHE[key] = build_nc(x.shape[0] // N_CORES)
    nc = _NC_CACHE[key]

    wt, biasrow, ones, sfull = _host_prep(x, system_id, W, b)

    n_rows = x.shape[0] // N_CORES
    n_tiles = n_rows // SUB_B
    in_maps = []
    for core in range(N_CORES):
        lo, hi = core * n_rows, (core + 1) * n_rows
        xT_shard = np.ascontiguousarray(x[lo:hi].T)  # [D, n_rows]
        sid_shard = np.ascontiguousarray(
            system_id[lo:hi].astype(np.float32).reshape(n_tiles, SUB_B).T
        )  # [128, n_tiles]
        in_maps.append(
            {
                "xT": xT_shard,
                "sid": sid_shard,
                "wt": wt,
                "biasrow": biasrow,
                "ones": ones,
                "sfull": sfull,
            }
        )

    res = run_bass_kernel_spmd(nc, in_maps, core_ids=list(range(N_CORES)))
    out = np.concatenate([res.results[i]["out"] for i in range(N_CORES)], axis=0)
    return out.astype(np.float32)
